# revision 13
# baseline (speedup 1.0000x reference)
"""Trainium2 Bass kernel for nn_ContrastiveLoss.

Full inputs: video [16, 4096, 256] f32, audio [16, 4096, 256] f32,
mask [16, 4096] int32. Output: scalar loss, shape (1,) f32.

Sharding: data-parallel over batch B=16 -> 2 samples per core on 8 cores.
Each core computes its partial (unnormalized) loss on-device; host sums the
8 partials and divides by B.

Per-core algorithm (all on device):
  idx_s   = argmax(mask[s])                  (first max, via combined-key max)
  v_anc   = video[s, idx_s] / max(||.||,eps) (indirect-DMA gather + normalize)
  a_anc   = audio[s, idx_s] / max(||.||,eps)
  pos_s   = <v_anc, a_anc>/TEMP
  For every row t: dot(video[s,t], a_anc) via DVE tensor_tensor_reduce and
  ||video[s,t]||^2 via ACT Square+accum (one streaming pass, overlapped with
  DMA); likewise audio rows against v_anc.
  S = sum_t exp(dot/max(norm,eps)/TEMP); neg = S - exp(pos)  (== sum over
  t != idx since the idx term equals exp(pos) exactly).
  partial = sum_s [ -pos_s + 0.5*ln(neg_a2v) + 0.5*ln(neg_v2a) ]
"""

import numpy as np

import concourse.bass as bass


class _StageDone(Exception):
    pass

import concourse.mybir as mybir
from concourse import bacc
from concourse.bass_utils import run_bass_kernel_spmd
from concourse.tile import TileContext

B, T, D = 16, 4096, 256
NCORES = 8
SPC = B // NCORES  # samples per core
TEMP = 0.07
EPS = 1e-8
P = 128
TPS = T // P  # 32 sub-tiles of 128 rows per sample
G = 8  # sub-tiles per DMA (1 MiB transfers)
NSTREAM = 2 * SPC  # (video,audio) x samples

F32 = mybir.dt.float32
I32 = mybir.dt.int32
AF = mybir.ActivationFunctionType
ALU = mybir.AluOpType
AX = mybir.AxisListType


def _build(stage="full"):
    nc = bacc.Bacc(trn_type="TRN2", debug=False)

    video = nc.dram_tensor("video", [SPC, T, D], F32, kind="ExternalInput")
    audio = nc.dram_tensor("audio", [SPC, T, D], F32, kind="ExternalInput")
    mask = nc.dram_tensor("mask", [SPC, T], I32, kind="ExternalInput")
    out = nc.dram_tensor("out", [1, 1], F32, kind="ExternalOutput")

    # Constants embedded in the NEFF.
    jj = np.arange(TPS, dtype=np.float32)
    pp = np.arange(P, dtype=np.float32)[:, None]
    revt_np = 4095.0 - (pp * TPS + jj[None, :])  # [128, 32]
    revt_np = np.tile(revt_np, (1, SPC)).astype(np.float32)  # [128, 64]
    revt_d = nc.inline_tensor(revt_np, name="revt")
    ident_d = nc.inline_tensor(np.eye(P, dtype=np.float32), name="ident")
    ones_d = nc.inline_tensor(np.ones((P, 1), dtype=np.float32), name="ones")
    srow_d = nc.inline_tensor(
        (np.arange(SPC, dtype=np.int32) * T)[:, None], name="srow"
    )
    # sel[:, s*128:(s+1)*128] is a [SPC, 128] all-ones-in-row-s selector used
    # to broadcast anchor row s across 128 partitions via PE matmul.
    sel_np = np.zeros((SPC, SPC * P), dtype=np.float32)
    for s in range(SPC):
        sel_np[s, s * P : (s + 1) * P] = 1.0
    sel_d = nc.inline_tensor(sel_np, name="sel")

    vflat = video[:].rearrange("s t d -> (s t) d")
    aflat = audio[:].rearrange("s t d -> (s t) d")
    vtiles = video[:].rearrange("s (k p) d -> s p k d", p=P)  # [2,128,32,256]
    atiles = audio[:].rearrange("s (k p) d -> s p k d", p=P)
    mtile = mask[:].rearrange("s (p j) -> p s j", p=P)  # [128, 2, 32]

    with TileContext(nc) as tc:
      try:
        with (
            tc.tile_pool(name="sb", bufs=1) as sb,
            tc.tile_pool(name="data", bufs=4) as data,
            tc.tile_pool(name="scr", bufs=2) as scr,
            tc.tile_pool(name="ps", bufs=2, space="PSUM") as ps,
            tc.tile_pool(name="ps1", bufs=1, space="PSUM") as ps1,
        ):
            def st(shape, tag, dtype=F32, pool=sb):
                return pool.tile(shape, dtype, tag=tag, name=tag)

            # ---- constants to SBUF ----
            revt = st([P, SPC * TPS], "revt")
            ident = st([P, P], "ident")
            ones = st([P, 1], "ones")
            srow = st([SPC, 1], "srow", dtype=I32)
            sel = st([SPC, SPC * P], "sel")
            nc.sync.dma_start(out=revt[:], in_=revt_d[:])
            nc.sync.dma_start(out=ident[:], in_=ident_d[:])
            nc.sync.dma_start(out=ones[:], in_=ones_d[:])
            nc.sync.dma_start(out=srow[:], in_=srow_d[:])
            nc.sync.dma_start(out=sel[:], in_=sel_d[:])

            warm = st([1, 1], "warm")
            nc.scalar.activation(out=warm[:], in_=ones[0:1, :], func=AF.Sqrt)

            # ---- Stage A: argmax of mask per sample ----
            mf = st([P, SPC * TPS], "mf")
            mf3 = mf[:].rearrange("p (s j) -> p s j", s=SPC)
            nc.gpsimd.dma_start(out=mf3, in_=mtile)  # int32 -> f32 cast
            comb = st([P, SPC * TPS], "comb")
            nc.vector.scalar_tensor_tensor(
                out=comb[:], in0=mf[:], scalar=4096.0, in1=revt[:],
                op0=ALU.mult, op1=ALU.add,
            )
            pmax = st([P, SPC], "pmax")
            for s in range(SPC):
                nc.vector.reduce_max(
                    out=pmax[:, s : s + 1],
                    in_=comb[:, s * TPS : (s + 1) * TPS],
                    axis=AX.X,
                )
            pmax_t = ps1.tile([SPC, P], F32, tag="pmax_t", name="pmax_t")
            nc.tensor.transpose(out=pmax_t[:], in_=pmax[:], identity=ident[:])
            Mf = st([SPC, 1], "Mf")
            nc.vector.reduce_max(out=Mf[:], in_=pmax_t[:], axis=AX.X)
            Mi = st([SPC, 1], "Mi", dtype=I32)
            nc.vector.tensor_copy(out=Mi[:], in_=Mf[:])
            # idx = 4095 - (M & 4095); gidx = idx + s*T
            ridx = st([SPC, 1], "ridx", dtype=I32)
            nc.vector.tensor_scalar(
                out=ridx[:], in0=Mi[:], scalar1=4095, scalar2=None,
                op0=ALU.bitwise_and,
            )
            idx = st([SPC, 1], "idx", dtype=I32)
            nc.vector.tensor_scalar(
                out=idx[:], in0=ridx[:], scalar1=-1, scalar2=4095,
                op0=ALU.mult, op1=ALU.add,
            )
            gidx = st([SPC, 1], "gidx", dtype=I32)
            nc.vector.tensor_tensor(
                out=gidx[:], in0=idx[:], in1=srow[:], op=ALU.add
            )

            if stage == "A":
                dbg = st([1, 1], "dbg")
                nc.vector.tensor_copy(out=dbg[:], in_=gidx[0:1, :])
                nc.sync.dma_start(out=out[:], in_=dbg[:])
            if stage == "A":
                raise _StageDone(nc)

            # ---- Stage B: gather + normalize anchors, pos ----
            anc_v = st([SPC, D], "anc_v")
            anc_a = st([SPC, D], "anc_a")
            nc.gpsimd.indirect_dma_start(
                out=anc_v[:], out_offset=None, in_=vflat,
                in_offset=bass.IndirectOffsetOnAxis(ap=gidx[:, :1], axis=0),
            )
            nc.gpsimd.indirect_dma_start(
                out=anc_a[:], out_offset=None, in_=aflat,
                in_offset=bass.IndirectOffsetOnAxis(ap=gidx[:, :1], axis=0),
            )
            if stage == "B1":
                nc.sync.dma_start(out=out[:], in_=anc_v[0:1, 0:1])
                raise _StageDone(nc)
            scr2 = st([SPC, D], "scr2")
            scr3 = st([SPC, D], "scr3")
            ssv = st([SPC, 1], "ssv")
            ssa = st([SPC, 1], "ssa")
            nc.vector.scalar_tensor_tensor(
                out=scr2[:], in0=anc_v[:], scalar=1.0, in1=anc_v[:],
                op0=ALU.mult, op1=ALU.mult, accum_out=ssv[:],
            )
            nc.vector.scalar_tensor_tensor(
                out=scr3[:], in0=anc_a[:], scalar=1.0, in1=anc_a[:],
                op0=ALU.mult, op1=ALU.mult, accum_out=ssa[:],
            )
            if stage == "B2":
                nc.sync.dma_start(out=out[:], in_=ssv[0:1, :])
                raise _StageDone(nc)
            nv = st([SPC, 1], "nv")
            na = st([SPC, 1], "na")
            nc.scalar.activation(out=nv[:], in_=ssv[:], func=AF.Sqrt)
            nc.scalar.activation(out=na[:], in_=ssa[:], func=AF.Sqrt)
            nc.vector.tensor_scalar_max(out=nv[:], in0=nv[:], scalar1=EPS)
            nc.vector.tensor_scalar_max(out=na[:], in0=na[:], scalar1=EPS)
            if stage == "B3":
                nc.sync.dma_start(out=out[:], in_=nv[0:1, :])
                raise _StageDone(nc)
            rv = st([SPC, 1], "rv")
            ra = st([SPC, 1], "ra")
            nc.vector.reciprocal(out=rv[:], in_=nv[:])
            nc.vector.reciprocal(out=ra[:], in_=na[:])
            if stage == "B4":
                nc.sync.dma_start(out=out[:], in_=rv[0:1, :])
                raise _StageDone(nc)
            anc_vn = st([SPC, D], "anc_vn")
            anc_an = st([SPC, D], "anc_an")
            nc.scalar.mul(out=anc_vn[:], in_=anc_v[:], mul=rv[:, :1])
            nc.scalar.mul(out=anc_an[:], in_=anc_a[:], mul=ra[:, :1])
            if stage == "B5":
                nc.sync.dma_start(out=out[:], in_=anc_vn[0:1, 0:1])
                raise _StageDone(nc)
            pos = st([SPC, 1], "pos")
            scr4 = st([SPC, D], "scr4")
            nc.vector.scalar_tensor_tensor(
                out=scr4[:], in0=anc_vn[:], scalar=1.0 / TEMP, in1=anc_an[:],
                op0=ALU.mult, op1=ALU.mult, accum_out=pos[:],
            )

            if stage == "B":
                nc.sync.dma_start(out=out[:], in_=pos[0:1, :])
                raise _StageDone(nc)

            # ---- Stage C: broadcast anchors across partitions ----
            # video rows pair with the audio anchor; audio rows with video's.
            bcast = []  # stream i = s*2 + (0 video, 1 audio)
            for s in range(SPC):
                bv = st([P, D], f"bv{s}")
                ba = st([P, D], f"ba{s}")
                bv_ps = ps.tile([P, D], F32, tag="bc_ps", name="bv_ps")
                ba_ps = ps.tile([P, D], F32, tag="bc_ps", name="ba_ps")
                nc.tensor.matmul(
                    out=bv_ps[:], lhsT=sel[:, s * P : (s + 1) * P],
                    rhs=anc_an[:], start=True, stop=True,
                )
                nc.tensor.matmul(
                    out=ba_ps[:], lhsT=sel[:, s * P : (s + 1) * P],
                    rhs=anc_vn[:], start=True, stop=True,
                )
                nc.vector.tensor_copy(out=bv[:], in_=bv_ps[:])
                nc.vector.tensor_copy(out=ba[:], in_=ba_ps[:])
                bcast += [bv, ba]

            dots = st([P, NSTREAM * TPS], "dots")
            ssq = st([P, NSTREAM * TPS], "ssq")
            # per-stream count of sumsq tiles handled by DVE bn_stats
            DCNT = [15, 9, 6, 6]
            DOFF = [0, 15, 24, 30]  # running offset into bnraw slots
            NDVE = sum(DCNT)  # 36
            bnraw = st([P, NDVE * 6], "bnraw")

            if stage == "C":
                dbg2 = st([1, 1], "dbg2")
                nc.vector.reduce_sum(out=dbg2[:], in_=bcast[3][0:1, :], axis=AX.X)
                nc.sync.dma_start(out=out[:], in_=dbg2[:])
                raise _StageDone(nc)

            # ---- Stage D: main streaming pass ----
            for s in range(SPC):
                for si, tiles_ap in ((0, vtiles), (1, atiles)):
                    i = s * 2 + si
                    bc = bcast[i]
                    for g in range(TPS // G):
                        big = data.tile([P, G, D], F32, tag="big", name="big")
                        nc.sync.dma_start(
                            out=big[:], in_=tiles_ap[s, :, g * G : (g + 1) * G, :]
                        )
                        for kk in range(G):
                            k = g * G + kk
                            col = i * TPS + k
                            dscr = scr.tile([P, D], F32, tag="dscr", name="dscr")
                            nc.vector.scalar_tensor_tensor(
                                out=dscr[:], in0=big[:, kk, :], scalar=1.0,
                                in1=bc[:], op0=ALU.mult, op1=ALU.mult,
                                accum_out=dots[:, col : col + 1],
                            )
                            psc = ps.tile([P, D], F32, tag="psc", name="psc")
                            nc.scalar.activation(
                                out=psc[:], in_=big[:, kk, :], func=AF.Square,
                                accum_out=ssq[:, col : col + 1],
                            )

            if stage == "D":
                dbg3 = st([1, 1], "dbg3")
                nc.vector.reduce_sum(out=dbg3[:], in_=dots[0:1, :], axis=AX.X)
                nc.sync.dma_start(out=out[:], in_=dbg3[:])
                raise _StageDone(nc)

            # ---- Stage E: epilogue ----
            W = NSTREAM * TPS  # 128
            nall = st([P, W], "nall")
            nc.scalar.activation(out=nall[:], in_=ssq[:], func=AF.Sqrt)
            nc.vector.tensor_scalar_max(out=nall[:], in0=nall[:], scalar1=EPS)
            rall = st([P, W], "rall")
            nc.vector.reciprocal(out=rall[:], in_=nall[:])
            sim = st([P, W], "sim")
            nc.vector.tensor_tensor(
                out=sim[:], in0=dots[:], in1=rall[:], op=ALU.mult
            )
            eall = st([P, W], "eall")
            Scols = st([P, NSTREAM], "Scols")
            for i in range(NSTREAM):
                nc.scalar.activation(
                    out=eall[:, i * TPS : (i + 1) * TPS],
                    in_=sim[:, i * TPS : (i + 1) * TPS],
                    func=AF.Exp, scale=1.0 / TEMP,
                    accum_out=Scols[:, i : i + 1],
                )
            SP = ps1.tile([1, NSTREAM], F32, tag="SP", name="SP")
            nc.tensor.matmul(
                out=SP[:], lhsT=ones[:], rhs=Scols[:], start=True, stop=True
            )
            pos_t = ps1.tile([1, SPC], F32, tag="pos_t", name="pos_t")
            nc.tensor.transpose(
                out=pos_t[:], in_=pos[:], identity=ident[:SPC, :SPC]
            )
            # epi layout: [0:4] S per stream, [4:6] pos, [6:8] exp(pos)
            epi = st([1, 8], "epi")
            nc.vector.tensor_copy(out=epi[:, 0:NSTREAM], in_=SP[:])
            nc.vector.tensor_copy(out=epi[:, 4 : 4 + SPC], in_=pos_t[:, 0:SPC])
            nc.scalar.activation(
                out=epi[:, 6 : 6 + SPC], in_=epi[:, 4 : 4 + SPC], func=AF.Exp
            )
            negs = st([1, NSTREAM], "negs")
            for s in range(SPC):
                nc.vector.tensor_scalar(
                    out=negs[:, 2 * s : 2 * s + 2],
                    in0=epi[:, 2 * s : 2 * s + 2],
                    scalar1=epi[:, 6 + s : 7 + s],
                    scalar2=None,
                    op0=ALU.subtract,
                )
            lns = st([1, NSTREAM], "lns")
            nc.scalar.activation(out=lns[:], in_=negs[:], func=AF.Ln)
            lsum = st([1, 1], "lsum")
            nc.vector.reduce_sum(out=lsum[:], in_=lns[:], axis=AX.X)
            psum2 = st([1, 1], "psum2")
            nc.vector.reduce_sum(
                out=psum2[:], in_=epi[:, 4 : 4 + SPC], axis=AX.X
            )
            res = st([1, 1], "res")
            nc.vector.scalar_tensor_tensor(
                out=res[:], in0=lsum[:], scalar=0.5, in1=psum2[:],
                op0=ALU.mult, op1=ALU.subtract,
            )
            nc.sync.dma_start(out=out[:], in_=res[:])
      except _StageDone:
        pass

    nc.compile()
    return nc


_NC = None


def _get_nc():
    global _NC
    if _NC is None:
        _NC = _build()
    return _NC


def kernel(video, audio, mask):
    video = np.asarray(video, dtype=np.float32)
    audio = np.asarray(audio, dtype=np.float32)
    mask = np.asarray(mask, dtype=np.int32)
    nc = _get_nc()
    in_maps = []
    for c in range(NCORES):
        sl = slice(c * SPC, (c + 1) * SPC)
        in_maps.append(
            {
                "video": np.ascontiguousarray(video[sl]),
                "audio": np.ascontiguousarray(audio[sl]),
                "mask": np.ascontiguousarray(mask[sl]),
            }
        )
    res = run_bass_kernel_spmd(nc, in_maps, core_ids=list(range(NCORES)))
    total = np.float64(0.0)
    for r in res.results:
        total += np.float64(r["out"][0, 0])
    return np.array([total / B], dtype=np.float32)
